# revision 21
# baseline (speedup 1.0000x reference)
"""Trainium2 Bass kernel for a ContextCrossAttnBlock (dense transformer block).

Strategy: data-parallel over batch B=8 across the 8 NeuronCores (one sample per
core); weights replicated. Everything on-chip is kept in "C-major" layout
([channels -> partitions, spatial L -> free]) so that the input [B,C,H,W] maps
directly onto SBUF and every linear / conv / attention matmul needs no input or
output transposes.

Per-core program highlights:
  - LayerNorm over channels (partition axis) via PE matmul-with-ones column
    sums, narrow per-position stats, PE broadcast back, DVE normalize.
    rstd computed as exp(-0.5*ln(var+eps)) to stay in the exp/ln ACT table set.
  - attention computed per (head, key-chunk): S^T chunk = kT.T @ qT on PE,
    exp on ACT straight out of PSUM, then AV with a ones-augmented V so the
    softmax denominators fall out of the same matmul; normalization deferred
    to a per-head reciprocal of the sums row. Heads padded 16->32 rows to sit
    on the PE tile_position grid.
  - the strided K-projection conv (3x3 s2) and the depthwise 3x3 conv are
    shifted matmuls on PE (depthwise via per-channel diagonal weights built
    on the host).
"""

import sys

import numpy as np

sys.path.insert(0, "/opt/trn_rl_repo")

import concourse.bacc as bacc
import concourse.bass as bass
import concourse.tile as tile
from concourse import mybir
from concourse.masks import make_identity

AF = mybir.ActivationFunctionType
OP = mybir.AluOpType
F32 = mybir.dt.float32

B = 8
C = 128
HH = 32
L = HH * HH  # 1024
HEADS = 8
HD = 16
FORE = 64
POST = 256
HID = 512
EPS = 1e-5

_CACHE = {}


# --------------------------------------------------------------------------
# host-side weight preprocessing
# --------------------------------------------------------------------------
def _prep_weights(params):
    p = {k: np.asarray(v, dtype=np.float32) for k, v in params.items()}
    w = {}

    for i, nm in enumerate(["ln_x1", "ln_x2", "ln_x3", "ln_x4"]):
        w[f"ln{i + 1}_w"] = p[f"{nm}_w"]
        w[f"ln{i + 1}_b"] = p[f"{nm}_b"]
    w["fore_ln_w"] = p["ln_fore_w"]
    w["fore_ln_b"] = p["ln_fore_b"]
    w["post_ln_w"] = p["ln_post_w"]
    w["post_ln_b"] = p["ln_post_b"]

    # Heads are padded 16 -> 32 rows so per-head matmul slices land on the
    # PE's 32-aligned tile_position grid. pad_cols maps a dense [*, 128]
    # q/k feature axis to a padded [*, 256] axis (head h -> cols 32h..32h+15;
    # rows 16..31 of each block are left zero so they contribute nothing).
    def pad_cols(a):
        out = np.zeros(a.shape[:-1] + (2 * C,), dtype=np.float32)
        for h in range(HEADS):
            out[..., 32 * h : 32 * h + HD] = a[..., h * HD : (h + 1) * HD]
        return out

    for stage, pre in ((1, "fore"), (2, "post")):
        qw = p[f"{pre}_qv_w"].copy()  # [256, 128]
        qb = p[f"{pre}_qv_b"].copy()  # [256]
        qw[:C] *= HD ** (-0.5)  # fold attention scale into q rows
        qb[:C] *= HD ** (-0.5)
        qwT = qw.T  # [128, 256]; cols 0:128 q, 128:256 v
        qp = pad_cols(qwT[:, :C])  # [128, 256]
        w[f"qv{stage}_wT"] = np.ascontiguousarray(
            np.concatenate([qp, qwT[:, C:]], axis=1))  # [128, 384]
        w[f"qv{stage}_b"] = np.concatenate([pad_cols(qb[:C]), qb[C:]])  # [384]
        # proj lhsT with head-padded input rows: [2(chunk), 128, 128],
        # chunk g row 32j+d = proj_w.T[(4g+j)*16+d, :], pad rows zero.
        pwT = p[f"{pre}_proj_w"].T  # [128(in), 128(out)]
        pp = np.zeros((2, C, C), dtype=np.float32)
        for h in range(HEADS):
            g, j = h // 4, h % 4
            pp[g, 32 * j : 32 * j + HD, :] = pwT[h * HD : (h + 1) * HD, :]
        w[f"proj{stage}_wT"] = pp
        w[f"proj{stage}_b"] = p[f"{pre}_proj_b"]

    # fore K-projection conv taps: [9, 64, 256] (tap, in, padded out)
    fk = p["fore_k_w"]  # [128, 64, 3, 3]
    w["fore_taps"] = np.ascontiguousarray(
        pad_cols(fk.transpose(2, 3, 1, 0).reshape(9, FORE, C)))
    w["fore_k_b"] = pad_cols(p["fore_k_b"])  # [256]

    # post ConvT taps: [4(kl), 2(cin chunk), 128, 256] (cin rows, padded out)
    pk = p["post_k_w"]  # [256, 128, 2, 2]
    w["post_taps"] = np.ascontiguousarray(
        pad_cols(pk.transpose(2, 3, 0, 1).reshape(4, 2, C, C)))
    w["post_k_b"] = pad_cols(p["post_k_b"])  # [256]

    for m in (1, 2):
        w[f"fc1_wT_{m}"] = np.ascontiguousarray(p[f"mlp{m}_fc1_w"].T)  # [128,512]
        w[f"fc1_b_{m}"] = p[f"mlp{m}_fc1_b"]
        dw = p[f"mlp{m}_dw_w"][:, 0]  # [512, 3, 3]
        diag = np.zeros((4, 9, C, C), dtype=np.float32)
        idx = np.arange(C)
        for mc in range(4):
            for t in range(9):
                diag[mc, t, idx, idx] = dw[mc * C : (mc + 1) * C, t // 3, t % 3]
        w[f"diag_{m}"] = diag.reshape(36, C, C)
        w[f"dw_b_{m}"] = p[f"mlp{m}_dw_b"]
        w[f"fc2_wT_{m}"] = np.ascontiguousarray(
            p[f"mlp{m}_fc2_w"].T.reshape(4, C, C)
        )  # [4(kchunk),128,128]
        w[f"fc2_b_{m}"] = p[f"mlp{m}_fc2_b"]
    w["mln_w"] = p["mlp2_ln_w"]
    w["mln_b"] = p["mlp2_ln_b"]

    # row-16 selector, replicated per 32-row block: broadcasts the sums row
    e32 = np.zeros((32, 32), dtype=np.float32)
    e32[HD, :] = 1.0
    w["E32"] = np.tile(e32, (4, 1))  # [128, 32]
    return w


_WEIGHT_SPECS = {
    "ln1_w": (C,), "ln1_b": (C,), "ln2_w": (C,), "ln2_b": (C,),
    "ln3_w": (C,), "ln3_b": (C,), "ln4_w": (C,), "ln4_b": (C,),
    "fore_ln_w": (FORE,), "fore_ln_b": (FORE,),
    "post_ln_w": (POST,), "post_ln_b": (POST,),
    "qv1_wT": (C, 3 * C), "qv1_b": (3 * C,),
    "proj1_wT": (2, C, C), "proj1_b": (C,),
    "qv2_wT": (C, 3 * C), "qv2_b": (3 * C,),
    "proj2_wT": (2, C, C), "proj2_b": (C,),
    "fore_taps": (9, FORE, 2 * C), "fore_k_b": (2 * C,),
    "post_taps": (4, 2, C, 2 * C), "post_k_b": (2 * C,),
    "fc1_wT_1": (C, HID), "fc1_b_1": (HID,),
    "diag_1": (36, C, C), "dw_b_1": (HID,),
    "fc2_wT_1": (4, C, C), "fc2_b_1": (C,),
    "fc1_wT_2": (C, HID), "fc1_b_2": (HID,),
    "diag_2": (36, C, C), "dw_b_2": (HID,),
    "fc2_wT_2": (4, C, C), "fc2_b_2": (C,),
    "mln_w": (HID,), "mln_b": (HID,),
    "E32": (C, 32),
}


# --------------------------------------------------------------------------
# program builder
# --------------------------------------------------------------------------
def _build_program():
    nc = bacc.Bacc(
        "TRN2",
        target_bir_lowering=False,
        debug=False,
        enable_asserts=True,
        num_devices=B,
    )
    D = {}

    def inp(name, shape):
        D[name] = nc.dram_tensor(name, list(shape), F32, kind="ExternalInput").ap()

    inp("x_in", (C, L))
    inp("fore_in", (FORE, 4 * L))
    inp("post_in", (POST, L // 4))
    for name, shape in _WEIGHT_SPECS.items():
        inp(name, shape)
    y_out = nc.dram_tensor("y_out", [C, L], F32, kind="ExternalOutput").ap()

    with tile.TileContext(nc) as tc:
        _emit(tc, nc, D, y_out)
    nc.compile()
    return nc


def _emit(tc, nc, D, y_out):
    from contextlib import ExitStack

    ctx = ExitStack()
    with ctx:
        consts = ctx.enter_context(tc.tile_pool(name="consts", bufs=1))
        work = ctx.enter_context(tc.tile_pool(name="work", bufs=2))
        psum = ctx.enter_context(tc.tile_pool(name="psum", bufs=2, space="PSUM"))

        mm = nc.tensor.matmul

        # ---------------- constants / weights in SBUF ----------------
        def cvec(name, n):
            # [n] dram vector -> [n,1] sbuf column
            t = consts.tile([n, 1], F32, name=name)
            nc.sync.dma_start(t, D[name].rearrange("(p o) -> p o", o=1))
            return t

        def cchunks(name, n):
            # [n] dram vector -> [128, n//128] (partition-chunked columns)
            k = n // C
            t = consts.tile([C, k], F32, name=name)
            nc.sync.dma_start(t, D[name].rearrange("(k p) -> p k", p=C))
            return t

        identity = consts.tile([C, C], F32)
        make_identity(nc, identity)

        inv128 = consts.tile([C, 1], F32)
        nc.vector.memset(inv128, 1.0 / 128.0)
        inv64 = consts.tile([FORE, 1], F32)
        nc.vector.memset(inv64, 1.0 / 64.0)
        inv256 = consts.tile([C, 1], F32)
        nc.vector.memset(inv256, 1.0 / 256.0)
        inv512 = consts.tile([C, 1], F32)
        nc.vector.memset(inv512, 1.0 / 512.0)
        ones_row = consts.tile([1, C], F32)
        nc.vector.memset(ones_row, 1.0)
        zero_t = consts.tile([C, 1], F32)
        nc.vector.memset(zero_t, 0.0)
        eps_t = consts.tile([1, 1], F32)
        nc.vector.memset(eps_t, EPS)

        ln_w = {i: cvec(f"ln{i}_w", C) for i in (1, 2, 3, 4)}
        ln_b = {i: cvec(f"ln{i}_b", C) for i in (1, 2, 3, 4)}
        fore_ln_w = cvec("fore_ln_w", FORE)
        fore_ln_b = cvec("fore_ln_b", FORE)
        post_ln_w = cchunks("post_ln_w", POST)
        post_ln_b = cchunks("post_ln_b", POST)

        qv_wT, qv_b, proj_wT, proj_b = {}, {}, {}, {}
        for s in (1, 2):
            t = consts.tile([C, 3 * C], F32, name=f"qv{s}_wT")
            nc.sync.dma_start(t, D[f"qv{s}_wT"])
            qv_wT[s] = t
            qv_b[s] = cchunks(f"qv{s}_b", 3 * C)
            t = consts.tile([C, 2, C], F32, name=f"proj{s}_wT")
            nc.sync.dma_start(t, D[f"proj{s}_wT"].rearrange("g p m -> p g m"))
            proj_wT[s] = t
            proj_b[s] = cvec(f"proj{s}_b", C)

        fore_taps = consts.tile([FORE, 9, 2 * C], F32)
        nc.sync.dma_start(fore_taps, D["fore_taps"].rearrange("t p m -> p t m"))
        fore_k_b = cchunks("fore_k_b", 2 * C)
        post_taps = consts.tile([C, 8, 2 * C], F32)
        nc.sync.dma_start(post_taps, D["post_taps"].rearrange("q c p m -> p (q c) m"))
        post_k_b = cchunks("post_k_b", 2 * C)

        fc1_wT, fc1_b, dw_b, fc2_wT, fc2_b = {}, {}, {}, {}, {}
        for m in (1, 2):
            t = consts.tile([C, HID], F32, name=f"fc1_wT_{m}")
            nc.sync.dma_start(t, D[f"fc1_wT_{m}"])
            fc1_wT[m] = t
            fc1_b[m] = cchunks(f"fc1_b_{m}", HID)
            dw_b[m] = cchunks(f"dw_b_{m}", HID)
            t = consts.tile([C, 4, C], F32, name=f"fc2_wT_{m}")
            nc.sync.dma_start(t, D[f"fc2_wT_{m}"].rearrange("k p m -> p k m"))
            fc2_wT[m] = t
            fc2_b[m] = cvec(f"fc2_b_{m}", C)
        mln_w = cchunks("mln_w", HID)
        mln_b = cchunks("mln_b", HID)
        E32 = consts.tile([C, 32], F32)
        nc.sync.dma_start(E32, D["E32"])

        # diag dw weights: one pool slot reused between the two mlps
        diag_pool = ctx.enter_context(tc.tile_pool(name="diagp", bufs=1))

        # ---------------- layernorm over channels (C-major) ----------------
        def fslc(ap, s, e):
            # slice [s:e) of the flattened free dims (s, e strip-aligned)
            fshape = ap.shape[1:]
            if len(fshape) == 1:
                return ap[:, s:e]
            a, bdim = fshape
            return ap[:, s // bdim : e // bdim, :]

        def ln_cmajor(chunks, inv_tile, outs, n, strip=1024):
            """chunks: list of (p, fn) with fn(st, wdt) -> AP [p, ...] strip view.
            outs: per chunk dict(w=, b=, dst_fn=, act=None)."""
            nch = len(chunks)
            for st in range(0, n, strip):
                wdt = min(strip, n - st)
                # per-position mean / mean-of-squares over channels
                sum_ps = psum.tile([1, wdt], F32, tag="big")
                sq_ps = psum.tile([1, wdt], F32, tag="big", name="sq_ps")
                for i, (p, fn) in enumerate(chunks):
                    ch = fn(st, wdt)
                    sq = work.tile(list(ch.shape), F32, tag="lnsq", bufs=2)
                    nc.vector.tensor_mul(sq, ch, ch)
                    for s in range(0, wdt, 512):
                        e = min(s + 512, wdt)
                        mm(sum_ps[0:1, s:e], inv_tile[:p, :], fslc(ch, s, e),
                           start=(i == 0), stop=(i == nch - 1))
                        mm(sq_ps[0:1, s:e], inv_tile[:p, :], fslc(sq, s, e),
                           start=(i == 0), stop=(i == nch - 1))
                mn = work.tile([1, wdt], F32, tag="lnn", bufs=4, name="mn")
                nc.scalar.copy(mn, sum_ps)
                msq = work.tile([1, wdt], F32, tag="lnn", bufs=4, name="msq")
                nc.scalar.copy(msq, sq_ps)
                mean2 = work.tile([1, wdt], F32, tag="lnn", bufs=4, name="mean2")
                nc.vector.tensor_mul(mean2, mn, mn)
                var = work.tile([1, wdt], F32, tag="lnn", bufs=4, name="var")
                nc.vector.tensor_sub(var, msq, mean2)
                # rstd = exp(-0.5 * ln(var + eps))  (stays in exp/ln table set)
                lnv = work.tile([1, wdt], F32, tag="lnn", bufs=4, name="lnv")
                nc.scalar.activation(lnv, var, AF.Ln, bias=eps_t)
                rstd = work.tile([1, wdt], F32, tag="lnn", bufs=4, name="rstd")
                nc.scalar.activation(rstd, lnv, AF.Exp, bias=zero_t[0:1, :],
                                     scale=-0.5)
                mr = work.tile([1, wdt], F32, tag="lnn", bufs=4, name="mr")
                nc.vector.tensor_mul(mr, mn, rstd)
                pmax = max(p for p, _ in chunks)
                aB = psum.tile([pmax, wdt], F32, tag="big")
                cB = psum.tile([pmax, wdt], F32, tag="big", name="cB")
                for s in range(0, wdt, 512):
                    e = min(s + 512, wdt)
                    mm(aB[:, s:e], ones_row[0:1, :pmax], rstd[:, s:e])
                    mm(cB[:, s:e], ones_row[0:1, :pmax], mr[:, s:e])
                for (p, fn), o in zip(chunks, outs):
                    ch = fn(st, wdt)
                    fshape = list(ch.shape[1:])
                    if len(fshape) == 2:
                        aBv = aB[:p, :].rearrange("p (a b) -> p a b", b=fshape[1])
                        cBv = cB[:p, :].rearrange("p (a b) -> p a b", b=fshape[1])
                    else:
                        aBv, cBv = aB[:p, :], cB[:p, :]
                    t1 = work.tile(list(ch.shape), F32, tag="lnt", bufs=2)
                    nc.vector.tensor_mul(t1, ch, aBv)
                    t2 = work.tile(list(ch.shape), F32, tag="lnt", bufs=2,
                                   name="t2")
                    nc.vector.tensor_sub(t2, t1, cBv)
                    dst = o["dst_fn"](st, wdt)
                    if o.get("act") is not None:
                        nc.scalar.activation(dst, t2, o["act"],
                                             scale=o["w"], bias=o["b"])
                    else:
                        nc.vector.tensor_scalar(dst, t2, o["w"], o["b"],
                                                op0=OP.mult, op1=OP.add)

        def sl2(t):
            return lambda st, wdt: t[:, st : st + wdt]

        # ---------------- attention ----------------
        def attention(stage, qTp, vT, kTp, dst):
            # qTp/kTp: two [128, L] tiles, head h at partitions 32*(h%4)..+15
            # of tile h//4 (k-side pad rows are exact zeros via host weights).
            # vaug per (m, h): [128, 32] = [v_h | ones | zeros] so each head's
            # AV output (16 o rows + 1 sums row + zeros) fills a full 32-row
            # block of o_ps at a tile_position-legal offset.
            vaug = work.tile([C, 8, HEADS, 32], F32, tag="vaug", bufs=1)
            nc.vector.memset(vaug[:, :, :, HD : HD + 1], 1.0)
            nc.vector.memset(vaug[:, :, :, HD + 1 : 32], 0.0)
            for m in range(8):
                vt_ps = psum.tile([C, C], F32, tag="big")
                nc.tensor.transpose(vt_ps, vT[:, m * C : (m + 1) * C], identity)
                nc.vector.tensor_copy(
                    vaug[:, m, :, 0:HD],
                    vt_ps.rearrange("p (h d) -> p h d", d=HD),
                )
            oTp = [work.tile([C, L], F32, tag="oT", bufs=2, name=f"oTp{g}")
                   for g in range(2)]
            for g in range(2):
                o_ps = psum.tile([C, L], F32, tag="o")
                for j in range(4):
                    h = 4 * g + j
                    for m in range(8):
                        st_ps = psum.tile([C, L], F32, tag="big")
                        lhsT = kTp[g][32 * j : 32 * j + 32, m * C : (m + 1) * C]
                        for s in (0, 512):
                            mm(st_ps[:, s : s + 512], lhsT,
                               qTp[g][32 * j : 32 * j + 32, s : s + 512],
                               tile_position=(32 * j, 0))
                        pt = work.tile([C, L], F32, tag="pt", bufs=3)
                        nc.scalar.activation(pt, st_ps, AF.Exp, bias=zero_t)
                        for s in (0, 512):
                            mm(o_ps[32 * j : 32 * j + 32, s : s + 512],
                               vaug[:, m, h, :], pt[:, s : s + 512],
                               start=(m == 0), stop=(m == 7),
                               tile_position=(0, 32 * j))
                nc.vector.tensor_copy(oTp[g], o_ps)
                # broadcast each head's sums row across its 32-row block,
                # then normalize in place: oTp = oTp / sums
                sb_ps = psum.tile([C, L], F32, tag="big", name="sb_ps")
                for j in range(4):
                    for s in (0, 512):
                        mm(sb_ps[32 * j : 32 * j + 32, s : s + 512],
                           E32[32 * j : 32 * j + 32, :],
                           oTp[g][32 * j : 32 * j + 32, s : s + 512],
                           tile_position=(32 * j, 32 * j))
                rT = work.tile([C, L], F32, tag="rT", bufs=1)
                nc.vector.reciprocal(rT, sb_ps)
                nc.vector.tensor_mul(oTp[g], oTp[g], rT)
            pr_ps = psum.tile([C, L], F32, tag="big")
            for s in (0, 512):
                for g in range(2):
                    mm(pr_ps[:, s : s + 512], proj_wT[stage][:, g, :],
                       oTp[g][:, s : s + 512],
                       start=(g == 0), stop=(g == 1))
            nc.scalar.activation(dst, pr_ps, AF.Identity, bias=proj_b[stage])

        # ---------------- qv projection ----------------
        def qv_proj(stage, nx):
            qTp = [work.tile([C, L], F32, tag="qT", bufs=2, name=f"qTp{g}")
                   for g in range(2)]
            vT = work.tile([C, L], F32, tag="vT", bufs=1)
            for mc, dstt in ((0, qTp[0]), (1, qTp[1]), (2, vT)):
                qv_ps = psum.tile([C, L], F32, tag="big")
                for s in (0, 512):
                    mm(qv_ps[:, s : s + 512],
                       qv_wT[stage][:, mc * C : (mc + 1) * C], nx[:, s : s + 512])
                nc.scalar.activation(dstt, qv_ps, AF.Identity,
                                     bias=qv_b[stage][:, mc : mc + 1])
            return qTp, vT

        # ---------------- mixffn ----------------
        def mixffn(m, nx, skip, add_src, dst):
            diag = diag_pool.tile([C, 36, C], F32, tag="diag")
            nc.sync.dma_start(diag[:, 0:18, :],
                              D[f"diag_{m}"].rearrange("t p m -> p t m")[:, 0:18, :])
            nc.sync.dma_start(diag[:, 18:36, :],
                              D[f"diag_{m}"].rearrange("t p m -> p t m")[:, 18:36, :])
            ax_tiles = []
            for mc in range(4):
                h1_ps = psum.tile([C, L], F32, tag="big")
                for s in (0, 512):
                    mm(h1_ps[:, s : s + 512],
                       fc1_wT[m][:, mc * C : (mc + 1) * C], nx[:, s : s + 512])
                pad = work.tile([C, 34, 34], F32, tag="h1pad", bufs=2)
                nc.gpsimd.memset(pad, 0.0)
                nc.scalar.activation(
                    pad[:, 1:33, 1:33],
                    h1_ps.rearrange("p (a b) -> p a b", a=HH),
                    AF.Identity, bias=fc1_b[m][:, mc : mc + 1])
                dw_ps = psum.tile([C, L], F32, tag="big")
                for t in range(9):
                    ky, kx = t // 3, t % 3
                    for half in (0, 1):
                        rhs = pad[:, ky + 16 * half : ky + 16 * half + 16,
                                  kx : kx + HH]
                        mm(dw_ps[:, half * 512 : half * 512 + 512],
                           diag[:, mc * 9 + t, :], rhs,
                           start=(t == 0), stop=(t == 8))
                if not skip:
                    ax = work.tile([C, L], F32, tag="ax", bufs=4)
                    nc.scalar.activation(ax, dw_ps, AF.Gelu,
                                         bias=dw_b[m][:, mc : mc + 1])
                    ax_tiles.append(ax)
                else:
                    ssb = work.tile([C, L], F32, tag="ax", bufs=4, name="ssb")
                    nc.vector.tensor_scalar(ssb, dw_ps, dw_b[m][:, mc : mc + 1],
                                            None, op0=OP.add)
                    sv = ssb.rearrange("p (a b) -> p a b", a=HH)
                    nc.vector.tensor_add(sv, sv, pad[:, 1:33, 1:33])
                    ax_tiles.append(ssb)
            if skip:
                # LN over the 512 hidden channels, gelu folded in, written
                # back in place (each strip is fully consumed before its
                # rewrite).
                outs = [dict(dst_fn=sl2(t), act=AF.Gelu,
                             w=mln_w[:, mc : mc + 1], b=mln_b[:, mc : mc + 1])
                        for mc, t in enumerate(ax_tiles)]
                ln_cmajor([(C, sl2(t)) for t in ax_tiles], inv512, outs, L)
            mlp_ps = psum.tile([C, L], F32, tag="big")
            for s in (0, 512):
                for kc in range(4):
                    mm(mlp_ps[:, s : s + 512], fc2_wT[m][:, kc, :],
                       ax_tiles[kc][:, s : s + 512],
                       start=(kc == 0), stop=(kc == 3))
            t = work.tile([C, L], F32, tag="mlpout", bufs=1)
            nc.scalar.activation(t, mlp_ps, AF.Identity, bias=fc2_b[m])
            nc.vector.tensor_add(dst, add_src, t)

        # ================== main flow ==================
        xcm = work.tile([C, L], F32, tag="resid", bufs=2, name="xcm")
        nc.sync.dma_start(xcm, D["x_in"])
        fore_pad = work.tile([FORE, 66, 66], F32, tag="fore_pad", bufs=1)
        nc.gpsimd.memset(fore_pad, 0.0)
        nc.sync.dma_start(fore_pad[:, 1:65, 1:65],
                          D["fore_in"].rearrange("p (a b) -> p a b", a=64))
        post_sb = work.tile([C, 2, L // 4], F32, tag="post_sb", bufs=1)
        nc.sync.dma_start(post_sb, D["post_in"].rearrange("(k p) n -> p k n", p=C))

        # ---- stage 1 ----
        nx1 = work.tile([C, L], F32, tag="nx", bufs=1)
        ln_cmajor([(C, sl2(xcm))], inv128,
                  [dict(dst_fn=sl2(nx1), w=ln_w[1], b=ln_b[1])], L)

        def fch(st, wdt):  # strip view of the padded fore interior
            r0 = st // 64
            return fore_pad[:, 1 + r0 : 1 + r0 + wdt // 64, 1:65]

        ln_cmajor([(FORE, fch)], inv64,
                  [dict(dst_fn=fch, w=fore_ln_w, b=fore_ln_b)], 4 * L)

        # fore K conv (3x3, stride 2, pad 1) -> head-padded kfT [2][128, 1024]
        kfT = [work.tile([C, L], F32, tag="kT", bufs=2, name=f"kfT{g}")
               for g in range(2)]
        for g in range(2):
            kf_ps = psum.tile([C, L], F32, tag="big", name="kf_ps")
            for t in range(9):
                ky, kx = t // 3, t % 3
                for half in (0, 1):
                    rhs = fore_pad[:, ky + 32 * half : ky + 32 * half + 32 : 2,
                                   kx : kx + 64 : 2]
                    mm(kf_ps[:, half * 512 : half * 512 + 512],
                       fore_taps[:, t, g * C : (g + 1) * C], rhs,
                       start=(t == 0), stop=(t == 8))
            nc.scalar.activation(kfT[g], kf_ps, AF.Identity,
                                 bias=fore_k_b[:, g : g + 1])

        qT1, vT1 = qv_proj(1, nx1)
        attn1 = work.tile([C, L], F32, tag="attn", bufs=1)
        attention(1, qT1, vT1, kfT, attn1)
        add1 = work.tile([C, L], F32, tag="resid", bufs=2, name="add1")
        nc.vector.tensor_add(add1, xcm, attn1)

        nx2 = work.tile([C, L], F32, tag="nx", bufs=1)
        ln_cmajor([(C, sl2(add1))], inv128,
                  [dict(dst_fn=sl2(nx2), w=ln_w[2], b=ln_b[2])], L)
        xt2 = work.tile([C, L], F32, tag="resid", bufs=2, name="xt2")
        mixffn(1, nx2, False, add1, xt2)

        # ---- stage 2 ----
        nx3 = work.tile([C, L], F32, tag="nx", bufs=1)
        ln_cmajor([(C, sl2(xt2))], inv128,
                  [dict(dst_fn=sl2(nx3), w=ln_w[3], b=ln_b[3])], L)

        np0 = work.tile([C, L // 4], F32, tag="npost", bufs=2)
        np1 = work.tile([C, L // 4], F32, tag="npost", bufs=2, name="np1")
        ln_cmajor(
            [(C, lambda st, w: post_sb[:, 0, st : st + w]),
             (C, lambda st, w: post_sb[:, 1, st : st + w])], inv256,
            [dict(dst_fn=sl2(np0), w=post_ln_w[:, 0:1], b=post_ln_b[:, 0:1]),
             dict(dst_fn=sl2(np1), w=post_ln_w[:, 1:2], b=post_ln_b[:, 1:2])],
            L // 4)

        # post ConvT (2x2, stride 2) -> head-padded kpT [2][128, 1024]
        kpT = [work.tile([C, L], F32, tag="kT", bufs=2, name=f"kpT{g}")
               for g in range(2)]
        for g in range(2):
            kpT_v = kpT[g].rearrange("p (y x) -> p y x", y=HH)
            for q in range(4):
                k_, l_ = q // 2, q % 2
                kp_ps = psum.tile([C, L // 4], F32, tag="big", name="kp_ps")
                for kc in range(2):
                    mm(kp_ps, post_taps[:, q * 2 + kc, g * C : (g + 1) * C],
                       np0 if kc == 0 else np1, start=(kc == 0), stop=(kc == 1))
                nc.scalar.activation(
                    kpT_v[:, k_ : HH : 2, l_ : HH : 2],
                    kp_ps.rearrange("p (i j) -> p i j", i=16),
                    AF.Identity, bias=post_k_b[:, g : g + 1])

        qT2, vT2 = qv_proj(2, nx3)
        attn2 = work.tile([C, L], F32, tag="attn", bufs=1, name="attn2")
        attention(2, qT2, vT2, kpT, attn2)
        add3 = work.tile([C, L], F32, tag="resid", bufs=2, name="add3")
        nc.vector.tensor_add(add3, xt2, attn2)

        nx4 = work.tile([C, L], F32, tag="nx", bufs=1)
        ln_cmajor([(C, sl2(add3))], inv128,
                  [dict(dst_fn=sl2(nx4), w=ln_w[4], b=ln_b[4])], L)
        y_sb = work.tile([C, L], F32, tag="resid", bufs=2, name="y_sb")
        mixffn(2, nx4, True, add3, y_sb)

        nc.sync.dma_start(y_out, y_sb)


# --------------------------------------------------------------------------
# public entry point
# --------------------------------------------------------------------------
def _get_program():
    if "nc" not in _CACHE:
        _CACHE["nc"] = _build_program()
    return _CACHE["nc"]


def make_in_maps(x, fore_x, post_x, params):
    x = np.asarray(x, dtype=np.float32)
    fore_x = np.asarray(fore_x, dtype=np.float32)
    post_x = np.asarray(post_x, dtype=np.float32)
    w = _prep_weights(params)
    in_maps = []
    for b in range(B):
        m = {
            "x_in": np.ascontiguousarray(x[b].reshape(C, L)),
            "fore_in": np.ascontiguousarray(fore_x[b].reshape(FORE, 4 * L)),
            "post_in": np.ascontiguousarray(post_x[b].reshape(POST, L // 4)),
        }
        m.update(w)
        in_maps.append(m)
    return in_maps


def kernel(x, fore_x, post_x, params, trace=False):
    from concourse.bass_utils import run_bass_kernel_spmd

    nc = _get_program()
    in_maps = make_in_maps(x, fore_x, post_x, params)
    res = run_bass_kernel_spmd(nc, in_maps, core_ids=list(range(B)),
                               trace=trace)
    if trace:
        kernel.last_results = res
    out = np.stack([r["y_out"].reshape(C, HH, HH) for r in res.results])
    return out


# revision 24
# speedup vs baseline: 1.4952x; 1.4952x over previous
"""Trainium2 Bass kernel for a ContextCrossAttnBlock (dense transformer block).

Strategy: data-parallel over batch B=8 across the 8 NeuronCores (one sample per
core); weights replicated. Everything on-chip is kept in "C-major" layout
([channels -> partitions, spatial L -> free]) so that the input [B,C,H,W] maps
directly onto SBUF and every linear / conv / attention matmul needs no input or
output transposes.

Per-core program highlights:
  - LayerNorm over channels (partition axis) via PE matmul-with-ones column
    sums, narrow per-position stats, PE broadcast back, DVE normalize.
    rstd computed as exp(-0.5*ln(var+eps)) to stay in the exp/ln ACT table set.
  - attention computed per (head, key-chunk): S^T chunk = kT.T @ qT on PE,
    exp on ACT straight out of PSUM, then AV with a ones-augmented V so the
    softmax denominators fall out of the same matmul; normalization deferred
    to a per-head reciprocal of the sums row. Heads padded 16->32 rows to sit
    on the PE tile_position grid.
  - the strided K-projection conv (3x3 s2) and the depthwise 3x3 conv are
    shifted matmuls on PE (depthwise via per-channel diagonal weights built
    on the host).
"""

import sys

import numpy as np

sys.path.insert(0, "/opt/trn_rl_repo")

import concourse.bacc as bacc
import concourse.bass as bass
import concourse.tile as tile
from concourse import mybir
from concourse.masks import make_identity

AF = mybir.ActivationFunctionType
OP = mybir.AluOpType
F32 = mybir.dt.float32
BF16 = mybir.dt.bfloat16

B = 8
C = 128
HH = 32
L = HH * HH  # 1024
HEADS = 8
HD = 16
FORE = 64
POST = 256
HID = 512
EPS = 1e-5

_CACHE = {}


# --------------------------------------------------------------------------
# host-side weight preprocessing
# --------------------------------------------------------------------------
def _prep_weights(params):
    p = {k: np.asarray(v, dtype=np.float32) for k, v in params.items()}
    w = {}

    for i, nm in enumerate(["ln_x1", "ln_x2", "ln_x3", "ln_x4"]):
        w[f"ln{i + 1}_w"] = p[f"{nm}_w"]
        w[f"ln{i + 1}_b"] = p[f"{nm}_b"]
    w["fore_ln_w"] = p["ln_fore_w"]
    w["fore_ln_b"] = p["ln_fore_b"]
    w["post_ln_w"] = p["ln_post_w"]
    w["post_ln_b"] = p["ln_post_b"]

    # Heads are padded 16 -> 32 rows so per-head matmul slices land on the
    # PE's 32-aligned tile_position grid. pad_cols maps a dense [*, 128]
    # q/k feature axis to a padded [*, 256] axis (head h -> cols 32h..32h+15;
    # rows 16..31 of each block are left zero so they contribute nothing).
    def pad_cols(a):
        out = np.zeros(a.shape[:-1] + (2 * C,), dtype=np.float32)
        for h in range(HEADS):
            out[..., 32 * h : 32 * h + HD] = a[..., h * HD : (h + 1) * HD]
        return out

    for stage, pre in ((1, "fore"), (2, "post")):
        qw = p[f"{pre}_qv_w"].copy()  # [256, 128]
        qb = p[f"{pre}_qv_b"].copy()  # [256]
        qw[:C] *= HD ** (-0.5)  # fold attention scale into q rows
        qb[:C] *= HD ** (-0.5)
        qwT = qw.T  # [128, 256]; cols 0:128 q, 128:256 v
        qp = pad_cols(qwT[:, :C])  # [128, 256]
        w[f"qv{stage}_wT"] = np.ascontiguousarray(
            np.concatenate([qp, qwT[:, C:]], axis=1))  # [128, 384]
        w[f"qv{stage}_b"] = np.concatenate([pad_cols(qb[:C]), qb[C:]])  # [384]
        # proj lhsT with head-padded input rows: [2(chunk), 128, 128],
        # chunk g row 32j+d = proj_w.T[(4g+j)*16+d, :], pad rows zero.
        pwT = p[f"{pre}_proj_w"].T  # [128(in), 128(out)]
        pp = np.zeros((2, C, C), dtype=np.float32)
        for h in range(HEADS):
            g, j = h // 4, h % 4
            pp[g, 32 * j : 32 * j + HD, :] = pwT[h * HD : (h + 1) * HD, :]
        w[f"proj{stage}_wT"] = pp
        w[f"proj{stage}_b"] = p[f"{pre}_proj_b"]

    # fore K-projection conv taps: [9, 64, 256] (tap, in, padded out)
    fk = p["fore_k_w"]  # [128, 64, 3, 3]
    w["fore_taps"] = np.ascontiguousarray(
        pad_cols(fk.transpose(2, 3, 1, 0).reshape(9, FORE, C)))
    w["fore_k_b"] = pad_cols(p["fore_k_b"])  # [256]

    # post ConvT taps: [4(kl), 2(cin chunk), 128, 256] (cin rows, padded out)
    pk = p["post_k_w"]  # [256, 128, 2, 2]
    w["post_taps"] = np.ascontiguousarray(
        pad_cols(pk.transpose(2, 3, 0, 1).reshape(4, 2, C, C)))
    w["post_k_b"] = pad_cols(p["post_k_b"])  # [256]

    for m in (1, 2):
        w[f"fc1_wT_{m}"] = np.ascontiguousarray(p[f"mlp{m}_fc1_w"].T)  # [128,512]
        w[f"fc1_b_{m}"] = p[f"mlp{m}_fc1_b"]
        dw = p[f"mlp{m}_dw_w"][:, 0]  # [512, 3, 3]
        diag = np.zeros((4, 9, C, C), dtype=np.float32)
        idx = np.arange(C)
        for mc in range(4):
            for t in range(9):
                diag[mc, t, idx, idx] = dw[mc * C : (mc + 1) * C, t // 3, t % 3]
        w[f"diag_{m}"] = diag.reshape(36, C, C)
        w[f"dw_b_{m}"] = p[f"mlp{m}_dw_b"]
        w[f"fc2_wT_{m}"] = np.ascontiguousarray(
            p[f"mlp{m}_fc2_w"].T.reshape(4, C, C)
        )  # [4(kchunk),128,128]
        w[f"fc2_b_{m}"] = p[f"mlp{m}_fc2_b"]
    w["mln_w"] = p["mlp2_ln_w"]
    w["mln_b"] = p["mlp2_ln_b"]

    # row-16 selector, replicated per 32-row block: broadcasts the sums row
    e32 = np.zeros((32, 32), dtype=np.float32)
    e32[HD, :] = 1.0
    w["E32"] = np.tile(e32, (4, 1))  # [128, 32]

    import ml_dtypes
    for k in _BF16_WEIGHTS:
        w[k] = w[k].astype(ml_dtypes.bfloat16)
    return w


_BF16_WEIGHTS = {
    "qv1_wT", "qv2_wT", "post_taps",
    "fc1_wT_1", "fc1_wT_2", "fc2_wT_1", "fc2_wT_2",
    "diag_1", "diag_2",
}

_WEIGHT_SPECS = {
    "ln1_w": (C,), "ln1_b": (C,), "ln2_w": (C,), "ln2_b": (C,),
    "ln3_w": (C,), "ln3_b": (C,), "ln4_w": (C,), "ln4_b": (C,),
    "fore_ln_w": (FORE,), "fore_ln_b": (FORE,),
    "post_ln_w": (POST,), "post_ln_b": (POST,),
    "qv1_wT": (C, 3 * C), "qv1_b": (3 * C,),
    "proj1_wT": (2, C, C), "proj1_b": (C,),
    "qv2_wT": (C, 3 * C), "qv2_b": (3 * C,),
    "proj2_wT": (2, C, C), "proj2_b": (C,),
    "fore_taps": (9, FORE, 2 * C), "fore_k_b": (2 * C,),
    "post_taps": (4, 2, C, 2 * C), "post_k_b": (2 * C,),
    "fc1_wT_1": (C, HID), "fc1_b_1": (HID,),
    "diag_1": (36, C, C), "dw_b_1": (HID,),
    "fc2_wT_1": (4, C, C), "fc2_b_1": (C,),
    "fc1_wT_2": (C, HID), "fc1_b_2": (HID,),
    "diag_2": (36, C, C), "dw_b_2": (HID,),
    "fc2_wT_2": (4, C, C), "fc2_b_2": (C,),
    "mln_w": (HID,), "mln_b": (HID,),
    "E32": (C, 32),
}


# --------------------------------------------------------------------------
# program builder
# --------------------------------------------------------------------------
def _build_program():
    nc = bacc.Bacc(
        "TRN2",
        target_bir_lowering=False,
        debug=False,
        enable_asserts=True,
        num_devices=B,
    )
    D = {}

    def inp(name, shape, dt=F32):
        D[name] = nc.dram_tensor(name, list(shape), dt, kind="ExternalInput").ap()

    inp("x_in", (C, L))
    inp("fore_in", (FORE, 4 * L))
    inp("post_in", (POST, L // 4))
    for name, shape in _WEIGHT_SPECS.items():
        inp(name, shape, BF16 if name in _BF16_WEIGHTS else F32)
    y_out = nc.dram_tensor("y_out", [C, L], F32, kind="ExternalOutput").ap()

    with tile.TileContext(nc) as tc:
        _emit(tc, nc, D, y_out)
    nc.compile()
    return nc


def _emit(tc, nc, D, y_out):
    from contextlib import ExitStack

    ctx = ExitStack()
    with ctx:
        consts = ctx.enter_context(tc.tile_pool(name="consts", bufs=1))
        work = ctx.enter_context(tc.tile_pool(name="work", bufs=2))
        psum = ctx.enter_context(tc.tile_pool(name="psum", bufs=2, space="PSUM"))

        mm = nc.tensor.matmul

        # ---------------- constants / weights in SBUF ----------------
        def cvec(name, n):
            # [n] dram vector -> [n,1] sbuf column
            t = consts.tile([n, 1], F32, name=name)
            nc.sync.dma_start(t, D[name].rearrange("(p o) -> p o", o=1))
            return t

        def cchunks(name, n):
            # [n] dram vector -> [128, n//128] (partition-chunked columns)
            k = n // C
            t = consts.tile([C, k], F32, name=name)
            nc.sync.dma_start(t, D[name].rearrange("(k p) -> p k", p=C))
            return t

        identity = consts.tile([C, C], BF16)
        make_identity(nc, identity)

        inv128 = consts.tile([C, 1], F32)
        nc.vector.memset(inv128, 1.0 / 128.0)
        inv64 = consts.tile([FORE, 1], F32)
        nc.vector.memset(inv64, 1.0 / 64.0)
        inv256 = consts.tile([C, 1], F32)
        nc.vector.memset(inv256, 1.0 / 256.0)
        inv512 = consts.tile([C, 1], F32)
        nc.vector.memset(inv512, 1.0 / 512.0)
        ones_row = consts.tile([1, C], F32)
        nc.vector.memset(ones_row, 1.0)
        zero_t = consts.tile([C, 1], F32)
        nc.vector.memset(zero_t, 0.0)
        eps_t = consts.tile([1, 1], F32)
        nc.vector.memset(eps_t, EPS)

        ln_w = {i: cvec(f"ln{i}_w", C) for i in (1, 2, 3, 4)}
        ln_b = {i: cvec(f"ln{i}_b", C) for i in (1, 2, 3, 4)}
        fore_ln_w = cvec("fore_ln_w", FORE)
        fore_ln_b = cvec("fore_ln_b", FORE)
        post_ln_w = cchunks("post_ln_w", POST)
        post_ln_b = cchunks("post_ln_b", POST)

        qv_wT, qv_b, proj_wT, proj_b = {}, {}, {}, {}
        for s in (1, 2):
            t = consts.tile([C, 3 * C], BF16, name=f"qv{s}_wT")
            nc.sync.dma_start(t, D[f"qv{s}_wT"])
            qv_wT[s] = t
            qv_b[s] = cchunks(f"qv{s}_b", 3 * C)
            t = consts.tile([C, 2, C], F32, name=f"proj{s}_wT")
            nc.sync.dma_start(t, D[f"proj{s}_wT"].rearrange("g p m -> p g m"))
            proj_wT[s] = t
            proj_b[s] = cvec(f"proj{s}_b", C)

        fore_taps = consts.tile([FORE, 9, 2 * C], F32)
        nc.sync.dma_start(fore_taps, D["fore_taps"].rearrange("t p m -> p t m"))
        fore_k_b = cchunks("fore_k_b", 2 * C)
        post_taps = consts.tile([C, 8, 2 * C], BF16)
        nc.sync.dma_start(post_taps, D["post_taps"].rearrange("q c p m -> p (q c) m"))
        post_k_b = cchunks("post_k_b", 2 * C)

        fc1_wT, fc1_b, dw_b, fc2_wT, fc2_b = {}, {}, {}, {}, {}
        for m in (1, 2):
            t = consts.tile([C, HID], BF16, name=f"fc1_wT_{m}")
            nc.sync.dma_start(t, D[f"fc1_wT_{m}"])
            fc1_wT[m] = t
            fc1_b[m] = cchunks(f"fc1_b_{m}", HID)
            dw_b[m] = cchunks(f"dw_b_{m}", HID)
            t = consts.tile([C, 4, C], BF16, name=f"fc2_wT_{m}")
            nc.sync.dma_start(t, D[f"fc2_wT_{m}"].rearrange("k p m -> p k m"))
            fc2_wT[m] = t
            fc2_b[m] = cvec(f"fc2_b_{m}", C)
        mln_w = cchunks("mln_w", HID)
        mln_b = cchunks("mln_b", HID)
        E32 = consts.tile([C, 32], F32)
        nc.sync.dma_start(E32, D["E32"])

        # diag dw weights: one pool slot reused between the two mlps
        diag_pool = ctx.enter_context(tc.tile_pool(name="diagp", bufs=1))

        # ---------------- layernorm over channels (C-major) ----------------
        def fslc(ap, s, e):
            # slice [s:e) of the flattened free dims (s, e strip-aligned)
            fshape = ap.shape[1:]
            if len(fshape) == 1:
                return ap[:, s:e]
            a, bdim = fshape
            return ap[:, s // bdim : e // bdim, :]

        def ln_cmajor(chunks, inv_tile, outs, n, strip=1024):
            """chunks: list of (p, fn) with fn(st, wdt) -> AP [p, ...] strip view.
            outs: per chunk dict(w=, b=, dst_fn=, act=None)."""
            nch = len(chunks)
            for st in range(0, n, strip):
                wdt = min(strip, n - st)
                # per-position mean / mean-of-squares over channels
                sum_ps = psum.tile([1, wdt], F32, tag="big")
                sq_ps = psum.tile([1, wdt], F32, tag="big", name="sq_ps")
                for i, (p, fn) in enumerate(chunks):
                    ch = fn(st, wdt)
                    sq = work.tile(list(ch.shape), F32, tag="lnsq", bufs=2)
                    nc.vector.tensor_mul(sq, ch, ch)
                    for s in range(0, wdt, 512):
                        e = min(s + 512, wdt)
                        mm(sum_ps[0:1, s:e], inv_tile[:p, :], fslc(ch, s, e),
                           start=(i == 0), stop=(i == nch - 1))
                        mm(sq_ps[0:1, s:e], inv_tile[:p, :], fslc(sq, s, e),
                           start=(i == 0), stop=(i == nch - 1))
                mn = work.tile([1, wdt], F32, tag="lnn", bufs=4, name="mn")
                nc.scalar.copy(mn, sum_ps)
                msq = work.tile([1, wdt], F32, tag="lnn", bufs=4, name="msq")
                nc.scalar.copy(msq, sq_ps)
                mean2 = work.tile([1, wdt], F32, tag="lnn", bufs=4, name="mean2")
                nc.vector.tensor_mul(mean2, mn, mn)
                var = work.tile([1, wdt], F32, tag="lnn", bufs=4, name="var")
                nc.vector.tensor_sub(var, msq, mean2)
                # rstd = exp(-0.5 * ln(var + eps))  (stays in exp/ln table set)
                lnv = work.tile([1, wdt], F32, tag="lnn", bufs=4, name="lnv")
                nc.scalar.activation(lnv, var, AF.Ln, bias=eps_t)
                rstd = work.tile([1, wdt], F32, tag="lnn", bufs=4, name="rstd")
                nc.scalar.activation(rstd, lnv, AF.Exp, bias=zero_t[0:1, :],
                                     scale=-0.5)
                mr = work.tile([1, wdt], F32, tag="lnn", bufs=4, name="mr")
                nc.vector.tensor_mul(mr, mn, rstd)
                pmax = max(p for p, _ in chunks)
                aB = psum.tile([pmax, wdt], F32, tag="big")
                cB = psum.tile([pmax, wdt], F32, tag="big", name="cB")
                for s in range(0, wdt, 512):
                    e = min(s + 512, wdt)
                    mm(aB[:, s:e], ones_row[0:1, :pmax], rstd[:, s:e])
                    mm(cB[:, s:e], ones_row[0:1, :pmax], mr[:, s:e])
                for (p, fn), o in zip(chunks, outs):
                    ch = fn(st, wdt)
                    fshape = list(ch.shape[1:])
                    if len(fshape) == 2:
                        aBv = aB[:p, :].rearrange("p (a b) -> p a b", b=fshape[1])
                        cBv = cB[:p, :].rearrange("p (a b) -> p a b", b=fshape[1])
                    else:
                        aBv, cBv = aB[:p, :], cB[:p, :]
                    t1 = work.tile(list(ch.shape), F32, tag="lnt", bufs=2)
                    nc.vector.tensor_mul(t1, ch, aBv)
                    t2 = work.tile(list(ch.shape), F32, tag="lnt", bufs=2,
                                   name="t2")
                    nc.vector.tensor_sub(t2, t1, cBv)
                    dst = o["dst_fn"](st, wdt)
                    if o.get("act") is not None:
                        nc.scalar.activation(dst, t2, o["act"],
                                             scale=o["w"], bias=o["b"])
                    else:
                        nc.vector.tensor_scalar(dst, t2, o["w"], o["b"],
                                                op0=OP.mult, op1=OP.add)

        def sl2(t):
            return lambda st, wdt: t[:, st : st + wdt]

        # ---------------- attention ----------------
        def attention(stage, qTp, vT, kTp, dst):
            # qTp/kTp: two [128, L] tiles, head h at partitions 32*(h%4)..+15
            # of tile h//4 (k-side pad rows are exact zeros via host weights).
            # vaug per (m, h): [128, 32] = [v_h | ones | zeros] so each head's
            # AV output (16 o rows + 1 sums row + zeros) fills a full 32-row
            # block of o_ps at a tile_position-legal offset.
            vaug = work.tile([C, 8, HEADS, 32], BF16, tag="vaug", bufs=1)
            nc.vector.memset(vaug[:, :, :, HD : HD + 1], 1.0)
            nc.vector.memset(vaug[:, :, :, HD + 1 : 32], 0.0)
            for m in range(8):
                vt_ps = psum.tile([C, C], BF16, tag="big")
                nc.tensor.transpose(vt_ps, vT[:, m * C : (m + 1) * C], identity)
                nc.vector.tensor_copy(
                    vaug[:, m, :, 0:HD],
                    vt_ps.rearrange("p (h d) -> p h d", d=HD),
                )
            oTp = [work.tile([C, L], F32, tag="oT", bufs=2, name=f"oTp{g}")
                   for g in range(2)]
            for g in range(2):
                o_ps = psum.tile([C, L], F32, tag="o")
                for j in range(4):
                    h = 4 * g + j
                    for m in range(8):
                        st_ps = psum.tile([C, L], F32, tag="big")
                        lhsT = kTp[g][32 * j : 32 * j + 32, m * C : (m + 1) * C]
                        for s in (0, 512):
                            mm(st_ps[:, s : s + 512], lhsT,
                               qTp[g][32 * j : 32 * j + 32, s : s + 512],
                               tile_position=(32 * j, 0))
                        pt = work.tile([C, L], BF16, tag="pt", bufs=3)
                        nc.scalar.activation(pt, st_ps, AF.Exp, bias=zero_t)
                        for s in (0, 512):
                            mm(o_ps[32 * j : 32 * j + 32, s : s + 512],
                               vaug[:, m, h, :], pt[:, s : s + 512],
                               start=(m == 0), stop=(m == 7),
                               tile_position=(0, 32 * j))
                nc.vector.tensor_copy(oTp[g], o_ps)
                # broadcast each head's sums row across its 32-row block,
                # then normalize in place: oTp = oTp / sums
                sb_ps = psum.tile([C, L], F32, tag="big", name="sb_ps")
                for j in range(4):
                    for s in (0, 512):
                        mm(sb_ps[32 * j : 32 * j + 32, s : s + 512],
                           E32[32 * j : 32 * j + 32, :],
                           oTp[g][32 * j : 32 * j + 32, s : s + 512],
                           tile_position=(32 * j, 32 * j))
                rT = work.tile([C, L], F32, tag="rT", bufs=1)
                nc.vector.reciprocal(rT, sb_ps)
                nc.vector.tensor_mul(oTp[g], oTp[g], rT)
            pr_ps = psum.tile([C, L], F32, tag="big")
            for s in (0, 512):
                for g in range(2):
                    mm(pr_ps[:, s : s + 512], proj_wT[stage][:, g, :],
                       oTp[g][:, s : s + 512],
                       start=(g == 0), stop=(g == 1))
            nc.scalar.activation(dst, pr_ps, AF.Identity, bias=proj_b[stage])

        # ---------------- qv projection ----------------
        def qv_proj(stage, nx):
            qTp = [work.tile([C, L], BF16, tag="qT", bufs=2, name=f"qTp{g}")
                   for g in range(2)]
            vT = work.tile([C, L], BF16, tag="vT", bufs=1)
            for mc, dstt in ((0, qTp[0]), (1, qTp[1]), (2, vT)):
                qv_ps = psum.tile([C, L], F32, tag="big")
                for s in (0, 512):
                    mm(qv_ps[:, s : s + 512],
                       qv_wT[stage][:, mc * C : (mc + 1) * C], nx[:, s : s + 512])
                nc.scalar.activation(dstt, qv_ps, AF.Identity,
                                     bias=qv_b[stage][:, mc : mc + 1])
            return qTp, vT

        # ---------------- mixffn ----------------
        def mixffn(m, nx, skip, add_src, dst):
            diag = diag_pool.tile([C, 36, C], BF16, tag="diag")
            nc.sync.dma_start(diag[:, 0:18, :],
                              D[f"diag_{m}"].rearrange("t p m -> p t m")[:, 0:18, :])
            nc.sync.dma_start(diag[:, 18:36, :],
                              D[f"diag_{m}"].rearrange("t p m -> p t m")[:, 18:36, :])
            ax_tiles = []
            for mc in range(4):
                h1_ps = psum.tile([C, L], F32, tag="big")
                for s in (0, 512):
                    mm(h1_ps[:, s : s + 512],
                       fc1_wT[m][:, mc * C : (mc + 1) * C], nx[:, s : s + 512])
                pad = work.tile([C, 34, 34], BF16, tag="h1pad", bufs=2)
                nc.gpsimd.memset(pad, 0.0)
                nc.scalar.activation(
                    pad[:, 1:33, 1:33],
                    h1_ps.rearrange("p (a b) -> p a b", a=HH),
                    AF.Identity, bias=fc1_b[m][:, mc : mc + 1])
                dw_ps = psum.tile([C, L], F32, tag="big")
                for t in range(9):
                    ky, kx = t // 3, t % 3
                    for half in (0, 1):
                        rhs = pad[:, ky + 16 * half : ky + 16 * half + 16,
                                  kx : kx + HH]
                        mm(dw_ps[:, half * 512 : half * 512 + 512],
                           diag[:, mc * 9 + t, :], rhs,
                           start=(t == 0), stop=(t == 8))
                if not skip:
                    ax = work.tile([C, L], BF16, tag="ax", bufs=4)
                    nc.scalar.activation(ax, dw_ps, AF.Gelu,
                                         bias=dw_b[m][:, mc : mc + 1])
                    ax_tiles.append(ax)
                else:
                    ssb = work.tile([C, L], F32, tag="ax", bufs=4, name="ssb")
                    nc.vector.tensor_scalar(ssb, dw_ps, dw_b[m][:, mc : mc + 1],
                                            None, op0=OP.add)
                    sv = ssb.rearrange("p (a b) -> p a b", a=HH)
                    nc.vector.tensor_add(sv, sv, pad[:, 1:33, 1:33])
                    ax_tiles.append(ssb)
            if skip:
                # LN over the 512 hidden channels, gelu folded in, cast to
                # bf16 chunks for fc2
                ax2 = [work.tile([C, L], BF16, tag="ax2", bufs=4,
                                 name=f"ax2_{mc}") for mc in range(4)]
                outs = [dict(dst_fn=sl2(ax2[mc]), act=AF.Gelu,
                             w=mln_w[:, mc : mc + 1], b=mln_b[:, mc : mc + 1])
                        for mc in range(4)]
                ln_cmajor([(C, sl2(t)) for t in ax_tiles], inv512, outs, L)
                ax_tiles = ax2
            mlp_ps = psum.tile([C, L], F32, tag="big")
            for s in (0, 512):
                for kc in range(4):
                    mm(mlp_ps[:, s : s + 512], fc2_wT[m][:, kc, :],
                       ax_tiles[kc][:, s : s + 512],
                       start=(kc == 0), stop=(kc == 3))
            t = work.tile([C, L], F32, tag="mlpout", bufs=1)
            nc.scalar.activation(t, mlp_ps, AF.Identity, bias=fc2_b[m])
            nc.vector.tensor_add(dst, add_src, t)

        # ================== main flow ==================
        xcm = work.tile([C, L], F32, tag="resid", bufs=2, name="xcm")
        nc.sync.dma_start(xcm, D["x_in"])
        fore_pad = work.tile([FORE, 66, 66], F32, tag="fore_pad", bufs=1)
        nc.gpsimd.memset(fore_pad, 0.0)
        nc.sync.dma_start(fore_pad[:, 1:65, 1:65],
                          D["fore_in"].rearrange("p (a b) -> p a b", a=64))
        post_sb = work.tile([C, 2, L // 4], F32, tag="post_sb", bufs=1)
        nc.sync.dma_start(post_sb, D["post_in"].rearrange("(k p) n -> p k n", p=C))

        # ---- stage 1 ----
        nx1 = work.tile([C, L], BF16, tag="nx", bufs=1)
        ln_cmajor([(C, sl2(xcm))], inv128,
                  [dict(dst_fn=sl2(nx1), w=ln_w[1], b=ln_b[1])], L)

        def fch(st, wdt):  # strip view of the padded fore interior
            r0 = st // 64
            return fore_pad[:, 1 + r0 : 1 + r0 + wdt // 64, 1:65]

        ln_cmajor([(FORE, fch)], inv64,
                  [dict(dst_fn=fch, w=fore_ln_w, b=fore_ln_b)], 4 * L)

        # fore K conv (3x3, stride 2, pad 1) -> head-padded kfT [2][128, 1024]
        kfT = [work.tile([C, L], BF16, tag="kT", bufs=2, name=f"kfT{g}")
               for g in range(2)]
        for g in range(2):
            kf_ps = psum.tile([C, L], F32, tag="big", name="kf_ps")
            for t in range(9):
                ky, kx = t // 3, t % 3
                for half in (0, 1):
                    rhs = fore_pad[:, ky + 32 * half : ky + 32 * half + 32 : 2,
                                   kx : kx + 64 : 2]
                    mm(kf_ps[:, half * 512 : half * 512 + 512],
                       fore_taps[:, t, g * C : (g + 1) * C], rhs,
                       start=(t == 0), stop=(t == 8))
            nc.scalar.activation(kfT[g], kf_ps, AF.Identity,
                                 bias=fore_k_b[:, g : g + 1])

        qT1, vT1 = qv_proj(1, nx1)
        attn1 = work.tile([C, L], F32, tag="attn", bufs=1)
        attention(1, qT1, vT1, kfT, attn1)
        add1 = work.tile([C, L], F32, tag="resid", bufs=2, name="add1")
        nc.vector.tensor_add(add1, xcm, attn1)

        nx2 = work.tile([C, L], BF16, tag="nx", bufs=1)
        ln_cmajor([(C, sl2(add1))], inv128,
                  [dict(dst_fn=sl2(nx2), w=ln_w[2], b=ln_b[2])], L)
        xt2 = work.tile([C, L], F32, tag="resid", bufs=2, name="xt2")
        mixffn(1, nx2, False, add1, xt2)

        # ---- stage 2 ----
        nx3 = work.tile([C, L], BF16, tag="nx", bufs=1)
        ln_cmajor([(C, sl2(xt2))], inv128,
                  [dict(dst_fn=sl2(nx3), w=ln_w[3], b=ln_b[3])], L)

        np0 = work.tile([C, L // 4], BF16, tag="npost", bufs=2)
        np1 = work.tile([C, L // 4], BF16, tag="npost", bufs=2, name="np1")
        ln_cmajor(
            [(C, lambda st, w: post_sb[:, 0, st : st + w]),
             (C, lambda st, w: post_sb[:, 1, st : st + w])], inv256,
            [dict(dst_fn=sl2(np0), w=post_ln_w[:, 0:1], b=post_ln_b[:, 0:1]),
             dict(dst_fn=sl2(np1), w=post_ln_w[:, 1:2], b=post_ln_b[:, 1:2])],
            L // 4)

        # post ConvT (2x2, stride 2) -> head-padded kpT [2][128, 1024]
        kpT = [work.tile([C, L], BF16, tag="kT", bufs=2, name=f"kpT{g}")
               for g in range(2)]
        for g in range(2):
            kpT_v = kpT[g].rearrange("p (y x) -> p y x", y=HH)
            for q in range(4):
                k_, l_ = q // 2, q % 2
                kp_ps = psum.tile([C, L // 4], F32, tag="big", name="kp_ps")
                for kc in range(2):
                    mm(kp_ps, post_taps[:, q * 2 + kc, g * C : (g + 1) * C],
                       np0 if kc == 0 else np1, start=(kc == 0), stop=(kc == 1))
                nc.scalar.activation(
                    kpT_v[:, k_ : HH : 2, l_ : HH : 2],
                    kp_ps.rearrange("p (i j) -> p i j", i=16),
                    AF.Identity, bias=post_k_b[:, g : g + 1])

        qT2, vT2 = qv_proj(2, nx3)
        attn2 = work.tile([C, L], F32, tag="attn", bufs=1, name="attn2")
        attention(2, qT2, vT2, kpT, attn2)
        add3 = work.tile([C, L], F32, tag="resid", bufs=2, name="add3")
        nc.vector.tensor_add(add3, xt2, attn2)

        nx4 = work.tile([C, L], BF16, tag="nx", bufs=1)
        ln_cmajor([(C, sl2(add3))], inv128,
                  [dict(dst_fn=sl2(nx4), w=ln_w[4], b=ln_b[4])], L)
        y_sb = work.tile([C, L], F32, tag="resid", bufs=2, name="y_sb")
        mixffn(2, nx4, True, add3, y_sb)

        nc.sync.dma_start(y_out, y_sb)


# --------------------------------------------------------------------------
# public entry point
# --------------------------------------------------------------------------
def _get_program():
    if "nc" not in _CACHE:
        _CACHE["nc"] = _build_program()
    return _CACHE["nc"]


def make_in_maps(x, fore_x, post_x, params):
    x = np.asarray(x, dtype=np.float32)
    fore_x = np.asarray(fore_x, dtype=np.float32)
    post_x = np.asarray(post_x, dtype=np.float32)
    w = _prep_weights(params)
    in_maps = []
    for b in range(B):
        m = {
            "x_in": np.ascontiguousarray(x[b].reshape(C, L)),
            "fore_in": np.ascontiguousarray(fore_x[b].reshape(FORE, 4 * L)),
            "post_in": np.ascontiguousarray(post_x[b].reshape(POST, L // 4)),
        }
        m.update(w)
        in_maps.append(m)
    return in_maps


def kernel(x, fore_x, post_x, params, trace=False):
    from concourse.bass_utils import run_bass_kernel_spmd

    nc = _get_program()
    in_maps = make_in_maps(x, fore_x, post_x, params)
    res = run_bass_kernel_spmd(nc, in_maps, core_ids=list(range(B)),
                               trace=trace)
    if trace:
        kernel.last_results = res
    out = np.stack([r["y_out"].reshape(C, HH, HH) for r in res.results])
    return out


# revision 26
# speedup vs baseline: 1.7286x; 1.1561x over previous
"""Trainium2 Bass kernel for a ContextCrossAttnBlock (dense transformer block).

Strategy: data-parallel over batch B=8 across the 8 NeuronCores (one sample per
core); weights replicated. Everything on-chip is kept in "C-major" layout
([channels -> partitions, spatial L -> free]) so that the input [B,C,H,W] maps
directly onto SBUF and every linear / conv / attention matmul needs no input or
output transposes.

Per-core program highlights:
  - LayerNorm over channels (partition axis) via PE matmul-with-ones column
    sums, narrow per-position stats, PE broadcast back, DVE normalize.
    rstd computed as exp(-0.5*ln(var+eps)) to stay in the exp/ln ACT table set.
  - attention computed per (head, key-chunk): S^T chunk = kT.T @ qT on PE,
    exp on ACT straight out of PSUM, then AV with a ones-augmented V so the
    softmax denominators fall out of the same matmul; normalization deferred
    to a per-head reciprocal of the sums row. Heads padded 16->32 rows to sit
    on the PE tile_position grid.
  - the strided K-projection conv (3x3 s2) and the depthwise 3x3 conv are
    shifted matmuls on PE (depthwise via per-channel diagonal weights built
    on the host).
"""

import sys

import numpy as np

sys.path.insert(0, "/opt/trn_rl_repo")

import concourse.bacc as bacc
import concourse.bass as bass
import concourse.tile as tile
from concourse import mybir
from concourse.masks import make_identity

AF = mybir.ActivationFunctionType
OP = mybir.AluOpType
F32 = mybir.dt.float32
BF16 = mybir.dt.bfloat16

B = 8
C = 128
HH = 32
L = HH * HH  # 1024
HEADS = 8
HD = 16
FORE = 64
POST = 256
HID = 512
EPS = 1e-5

_CACHE = {}


# --------------------------------------------------------------------------
# host-side weight preprocessing
# --------------------------------------------------------------------------
def _prep_weights(params):
    p = {k: np.asarray(v, dtype=np.float32) for k, v in params.items()}
    w = {}

    for i, nm in enumerate(["ln_x1", "ln_x2", "ln_x3", "ln_x4"]):
        w[f"ln{i + 1}_w"] = p[f"{nm}_w"]
        w[f"ln{i + 1}_b"] = p[f"{nm}_b"]
    w["fore_ln_w"] = p["ln_fore_w"]
    w["fore_ln_b"] = p["ln_fore_b"]
    w["post_ln_w"] = p["ln_post_w"]
    w["post_ln_b"] = p["ln_post_b"]

    # Heads are padded 16 -> 32 rows so per-head matmul slices land on the
    # PE's 32-aligned tile_position grid. pad_cols maps a dense [*, 128]
    # q/k feature axis to a padded [*, 256] axis (head h -> cols 32h..32h+15;
    # rows 16..31 of each block are left zero so they contribute nothing).
    def pad_cols(a):
        out = np.zeros(a.shape[:-1] + (2 * C,), dtype=np.float32)
        for h in range(HEADS):
            out[..., 32 * h : 32 * h + HD] = a[..., h * HD : (h + 1) * HD]
        return out

    for stage, pre in ((1, "fore"), (2, "post")):
        qw = p[f"{pre}_qv_w"].copy()  # [256, 128]
        qb = p[f"{pre}_qv_b"].copy()  # [256]
        qw[:C] *= HD ** (-0.5)  # fold attention scale into q rows
        qb[:C] *= HD ** (-0.5)
        qwT = qw.T  # [128, 256]; cols 0:128 q, 128:256 v
        qp = pad_cols(qwT[:, :C])  # [128, 256]
        w[f"qv{stage}_wT"] = np.ascontiguousarray(
            np.concatenate([qp, qwT[:, C:]], axis=1))  # [128, 384]
        w[f"qv{stage}_b"] = np.concatenate([pad_cols(qb[:C]), qb[C:]])  # [384]
        # proj lhsT with head-padded input rows: [2(chunk), 128, 128],
        # chunk g row 32j+d = proj_w.T[(4g+j)*16+d, :], pad rows zero.
        pwT = p[f"{pre}_proj_w"].T  # [128(in), 128(out)]
        pp = np.zeros((2, C, C), dtype=np.float32)
        for h in range(HEADS):
            g, j = h // 4, h % 4
            pp[g, 32 * j : 32 * j + HD, :] = pwT[h * HD : (h + 1) * HD, :]
        w[f"proj{stage}_wT"] = pp
        w[f"proj{stage}_b"] = p[f"{pre}_proj_b"]

    # fore K-projection conv taps: [9, 64, 256] (tap, in, padded out)
    fk = p["fore_k_w"]  # [128, 64, 3, 3]
    w["fore_taps"] = np.ascontiguousarray(
        pad_cols(fk.transpose(2, 3, 1, 0).reshape(9, FORE, C)))
    w["fore_k_b"] = pad_cols(p["fore_k_b"])  # [256]

    # post ConvT taps: [4(kl), 2(cin chunk), 128, 256] (cin rows, padded out)
    pk = p["post_k_w"]  # [256, 128, 2, 2]
    w["post_taps"] = np.ascontiguousarray(
        pad_cols(pk.transpose(2, 3, 0, 1).reshape(4, 2, C, C)))
    w["post_k_b"] = pad_cols(p["post_k_b"])  # [256]

    for m in (1, 2):
        w[f"fc1_wT_{m}"] = np.ascontiguousarray(p[f"mlp{m}_fc1_w"].T)  # [128,512]
        w[f"fc1_b_{m}"] = p[f"mlp{m}_fc1_b"]
        dw = p[f"mlp{m}_dw_w"][:, 0]  # [512, 3, 3]
        diag = np.zeros((4, 9, C, C), dtype=np.float32)
        idx = np.arange(C)
        for mc in range(4):
            for t in range(9):
                diag[mc, t, idx, idx] = dw[mc * C : (mc + 1) * C, t // 3, t % 3]
        w[f"diag_{m}"] = diag.reshape(36, C, C)
        w[f"dw_b_{m}"] = p[f"mlp{m}_dw_b"]
        w[f"fc2_wT_{m}"] = np.ascontiguousarray(
            p[f"mlp{m}_fc2_w"].T.reshape(4, C, C)
        )  # [4(kchunk),128,128]
        w[f"fc2_b_{m}"] = p[f"mlp{m}_fc2_b"]
    w["mln_w"] = p["mlp2_ln_w"]
    w["mln_b"] = p["mlp2_ln_b"]

    # row-16 selector, replicated per 32-row block: broadcasts the sums row
    e32 = np.zeros((32, 32), dtype=np.float32)
    e32[HD, :] = 1.0
    w["E32"] = np.tile(e32, (4, 1))  # [128, 32]

    import ml_dtypes
    for k in _BF16_WEIGHTS:
        w[k] = w[k].astype(ml_dtypes.bfloat16)
    return w


_BF16_WEIGHTS = {
    "qv1_wT", "qv2_wT", "post_taps", "fore_taps",
    "fc1_wT_1", "fc1_wT_2", "fc2_wT_1", "fc2_wT_2",
    "diag_1", "diag_2",
}

_WEIGHT_SPECS = {
    "ln1_w": (C,), "ln1_b": (C,), "ln2_w": (C,), "ln2_b": (C,),
    "ln3_w": (C,), "ln3_b": (C,), "ln4_w": (C,), "ln4_b": (C,),
    "fore_ln_w": (FORE,), "fore_ln_b": (FORE,),
    "post_ln_w": (POST,), "post_ln_b": (POST,),
    "qv1_wT": (C, 3 * C), "qv1_b": (3 * C,),
    "proj1_wT": (2, C, C), "proj1_b": (C,),
    "qv2_wT": (C, 3 * C), "qv2_b": (3 * C,),
    "proj2_wT": (2, C, C), "proj2_b": (C,),
    "fore_taps": (9, FORE, 2 * C), "fore_k_b": (2 * C,),
    "post_taps": (4, 2, C, 2 * C), "post_k_b": (2 * C,),
    "fc1_wT_1": (C, HID), "fc1_b_1": (HID,),
    "diag_1": (36, C, C), "dw_b_1": (HID,),
    "fc2_wT_1": (4, C, C), "fc2_b_1": (C,),
    "fc1_wT_2": (C, HID), "fc1_b_2": (HID,),
    "diag_2": (36, C, C), "dw_b_2": (HID,),
    "fc2_wT_2": (4, C, C), "fc2_b_2": (C,),
    "mln_w": (HID,), "mln_b": (HID,),
    "E32": (C, 32),
}


# --------------------------------------------------------------------------
# program builder
# --------------------------------------------------------------------------
def _build_program():
    nc = bacc.Bacc(
        "TRN2",
        target_bir_lowering=False,
        debug=False,
        enable_asserts=True,
        num_devices=B,
    )
    D = {}

    def inp(name, shape, dt=F32):
        D[name] = nc.dram_tensor(name, list(shape), dt, kind="ExternalInput").ap()

    inp("x_in", (C, L))
    inp("fore_in", (FORE, 4 * L))
    inp("post_in", (POST, L // 4))
    for name, shape in _WEIGHT_SPECS.items():
        inp(name, shape, BF16 if name in _BF16_WEIGHTS else F32)
    y_out = nc.dram_tensor("y_out", [C, L], F32, kind="ExternalOutput").ap()

    with tile.TileContext(nc) as tc:
        _emit(tc, nc, D, y_out)
    nc.compile()
    return nc


def _emit(tc, nc, D, y_out):
    from contextlib import ExitStack

    ctx = ExitStack()
    with ctx:
        consts = ctx.enter_context(tc.tile_pool(name="consts", bufs=1))
        work = ctx.enter_context(tc.tile_pool(name="work", bufs=2))
        psum = ctx.enter_context(tc.tile_pool(name="psum", bufs=2, space="PSUM"))

        mm = nc.tensor.matmul

        # ---------------- constants / weights in SBUF ----------------
        def cvec(name, n):
            # [n] dram vector -> [n,1] sbuf column
            t = consts.tile([n, 1], F32, name=name)
            nc.sync.dma_start(t, D[name].rearrange("(p o) -> p o", o=1))
            return t

        def cchunks(name, n):
            # [n] dram vector -> [128, n//128] (partition-chunked columns)
            k = n // C
            t = consts.tile([C, k], F32, name=name)
            nc.sync.dma_start(t, D[name].rearrange("(k p) -> p k", p=C))
            return t

        identity = consts.tile([C, C], BF16)
        make_identity(nc, identity)

        inv128 = consts.tile([C, 1], F32)
        nc.vector.memset(inv128, 1.0 / 128.0)
        inv64 = consts.tile([FORE, 1], F32)
        nc.vector.memset(inv64, 1.0 / 64.0)
        inv256 = consts.tile([C, 1], F32)
        nc.vector.memset(inv256, 1.0 / 256.0)
        inv512 = consts.tile([C, 1], F32)
        nc.vector.memset(inv512, 1.0 / 512.0)
        ones_row = consts.tile([1, C], BF16)
        nc.vector.memset(ones_row, 1.0)
        zero_t = consts.tile([C, 1], F32)
        nc.vector.memset(zero_t, 0.0)
        eps_t = consts.tile([1, 1], F32)
        nc.vector.memset(eps_t, EPS)

        ln_w = {i: cvec(f"ln{i}_w", C) for i in (1, 2, 3, 4)}
        ln_b = {i: cvec(f"ln{i}_b", C) for i in (1, 2, 3, 4)}
        fore_ln_w = cvec("fore_ln_w", FORE)
        fore_ln_b = cvec("fore_ln_b", FORE)
        post_ln_w = cchunks("post_ln_w", POST)
        post_ln_b = cchunks("post_ln_b", POST)

        qv_wT, qv_b, proj_wT, proj_b = {}, {}, {}, {}
        for s in (1, 2):
            t = consts.tile([C, 3 * C], BF16, name=f"qv{s}_wT")
            nc.sync.dma_start(t, D[f"qv{s}_wT"])
            qv_wT[s] = t
            qv_b[s] = cchunks(f"qv{s}_b", 3 * C)
            t = consts.tile([C, 2, C], F32, name=f"proj{s}_wT")
            nc.sync.dma_start(t, D[f"proj{s}_wT"].rearrange("g p m -> p g m"))
            proj_wT[s] = t
            proj_b[s] = cvec(f"proj{s}_b", C)

        fore_taps = consts.tile([FORE, 9, 2 * C], BF16)
        nc.sync.dma_start(fore_taps, D["fore_taps"].rearrange("t p m -> p t m"))
        fore_k_b = cchunks("fore_k_b", 2 * C)
        post_taps = consts.tile([C, 8, 2 * C], BF16)
        nc.sync.dma_start(post_taps, D["post_taps"].rearrange("q c p m -> p (q c) m"))
        post_k_b = cchunks("post_k_b", 2 * C)

        fc1_wT, fc1_b, dw_b, fc2_wT, fc2_b = {}, {}, {}, {}, {}
        for m in (1, 2):
            t = consts.tile([C, HID], BF16, name=f"fc1_wT_{m}")
            nc.sync.dma_start(t, D[f"fc1_wT_{m}"])
            fc1_wT[m] = t
            fc1_b[m] = cchunks(f"fc1_b_{m}", HID)
            dw_b[m] = cchunks(f"dw_b_{m}", HID)
            t = consts.tile([C, 4, C], BF16, name=f"fc2_wT_{m}")
            nc.sync.dma_start(t, D[f"fc2_wT_{m}"].rearrange("k p m -> p k m"))
            fc2_wT[m] = t
            fc2_b[m] = cvec(f"fc2_b_{m}", C)
        mln_w = cchunks("mln_w", HID)
        mln_b = cchunks("mln_b", HID)
        E32 = consts.tile([C, 32], F32)
        nc.sync.dma_start(E32, D["E32"])

        # diag dw weights: one pool slot reused between the two mlps
        diag_pool = ctx.enter_context(tc.tile_pool(name="diagp", bufs=1))

        # ---------------- layernorm over channels (C-major) ----------------
        def fslc(ap, s, e):
            # slice [s:e) of the flattened free dims (s, e strip-aligned)
            fshape = ap.shape[1:]
            if len(fshape) == 1:
                return ap[:, s:e]
            a, bdim = fshape
            return ap[:, s // bdim : e // bdim, :]

        def ln_cmajor(chunks, inv_tile, outs, n, strip=1024):
            """chunks: list of (p, fn) with fn(st, wdt) -> AP [p, ...] strip view.
            outs: per chunk dict(w=, b=, dst_fn=, act=None)."""
            nch = len(chunks)
            for st in range(0, n, strip):
                wdt = min(strip, n - st)
                # per-position mean / mean-of-squares over channels
                sum_ps = psum.tile([1, wdt], F32, tag="big", bufs=3)
                sq_ps = psum.tile([1, wdt], F32, tag="big", bufs=3, name="sq_ps")
                for i, (p, fn) in enumerate(chunks):
                    ch = fn(st, wdt)
                    sq = work.tile(list(ch.shape), F32, tag="lnsq", bufs=2)
                    nc.vector.tensor_mul(sq, ch, ch)
                    for s in range(0, wdt, 512):
                        e = min(s + 512, wdt)
                        mm(sum_ps[0:1, s:e], inv_tile[:p, :], fslc(ch, s, e),
                           start=(i == 0), stop=(i == nch - 1))
                        mm(sq_ps[0:1, s:e], inv_tile[:p, :], fslc(sq, s, e),
                           start=(i == 0), stop=(i == nch - 1))
                mn = work.tile([1, wdt], F32, tag="lnn", bufs=4, name="mn")
                nc.scalar.copy(mn, sum_ps)
                msq = work.tile([1, wdt], F32, tag="lnn", bufs=4, name="msq")
                nc.scalar.copy(msq, sq_ps)
                mean2 = work.tile([1, wdt], F32, tag="lnn", bufs=4, name="mean2")
                nc.vector.tensor_mul(mean2, mn, mn)
                var = work.tile([1, wdt], F32, tag="lnn", bufs=4, name="var")
                nc.vector.tensor_sub(var, msq, mean2)
                # rstd = exp(-0.5 * ln(var + eps))  (stays in exp/ln table set)
                lnv = work.tile([1, wdt], F32, tag="lnn", bufs=4, name="lnv")
                nc.scalar.activation(lnv, var, AF.Ln, bias=eps_t)
                rstd = work.tile([1, wdt], BF16, tag="lnn", bufs=4, name="rstd")
                nc.scalar.activation(rstd, lnv, AF.Exp, bias=zero_t[0:1, :],
                                     scale=-0.5)
                mr = work.tile([1, wdt], BF16, tag="lnn", bufs=4, name="mr")
                nc.vector.tensor_mul(mr, mn, rstd)
                pmax = max(p for p, _ in chunks)
                aB = psum.tile([pmax, wdt], F32, tag="big", bufs=3)
                cB = psum.tile([pmax, wdt], F32, tag="big", bufs=3, name="cB")
                for s in range(0, wdt, 512):
                    e = min(s + 512, wdt)
                    mm(aB[:, s:e], ones_row[0:1, :pmax], rstd[:, s:e])
                    mm(cB[:, s:e], ones_row[0:1, :pmax], mr[:, s:e])
                for (p, fn), o in zip(chunks, outs):
                    ch = fn(st, wdt)
                    fshape = list(ch.shape[1:])
                    if len(fshape) == 2:
                        aBv = aB[:p, :].rearrange("p (a b) -> p a b", b=fshape[1])
                        cBv = cB[:p, :].rearrange("p (a b) -> p a b", b=fshape[1])
                    else:
                        aBv, cBv = aB[:p, :], cB[:p, :]
                    t1 = work.tile(list(ch.shape), F32, tag="lnt", bufs=2)
                    nc.vector.tensor_mul(t1, ch, aBv)
                    t2 = work.tile(list(ch.shape), F32, tag="lnt", bufs=2,
                                   name="t2")
                    nc.vector.tensor_sub(t2, t1, cBv)
                    dst = o["dst_fn"](st, wdt)
                    if len(dst.shape) == 3 and len(t2.shape) == 2:
                        t2 = t2.rearrange("p (a b) -> p a b", b=dst.shape[-1])
                    if o.get("act") is not None:
                        nc.scalar.activation(dst, t2, o["act"],
                                             scale=o["w"], bias=o["b"])
                    else:
                        nc.vector.tensor_scalar(dst, t2, o["w"], o["b"],
                                                op0=OP.mult, op1=OP.add)

        def sl2(t):
            return lambda st, wdt: t[:, st : st + wdt]

        def evac(dst, ps, bias_col):
            # PSUM -> SBUF eviction with per-partition bias on DVE (no ACT
            # table-set traffic)
            nc.vector.tensor_scalar(dst, ps, bias_col, None, op0=OP.add)

        # ---------------- attention ----------------
        def attention(stage, qTp, vT, kTp, dst):
            # qTp/kTp: two [128, L] tiles, head h at partitions 32*(h%4)..+15
            # of tile h//4 (k-side pad rows are exact zeros via host weights).
            # vaug per (m, h): [128, 32] = [v_h | ones | zeros] so each head's
            # AV output (16 o rows + 1 sums row + zeros) fills a full 32-row
            # block of o_ps at a tile_position-legal offset.
            vaug = work.tile([C, 8, HEADS, 32], BF16, tag="vaug", bufs=1)
            nc.vector.memset(vaug[:, :, :, HD : HD + 1], 1.0)
            nc.vector.memset(vaug[:, :, :, HD + 1 : 32], 0.0)
            for m in range(8):
                vt_ps = psum.tile([C, C], BF16, tag="big", bufs=3)
                nc.tensor.transpose(vt_ps, vT[:, m * C : (m + 1) * C], identity)
                nc.vector.tensor_copy(
                    vaug[:, m, :, 0:HD],
                    vt_ps.rearrange("p (h d) -> p h d", d=HD),
                )
            oTp = [work.tile([C, L], F32, tag="oT", bufs=2, name=f"oTp{g}")
                   for g in range(2)]
            for g in range(2):
                o_ps = psum.tile([C, L], F32, tag="o", bufs=1)
                for j in range(4):
                    h = 4 * g + j
                    for m in range(8):
                        st_ps = psum.tile([C, L], F32, tag="big", bufs=3)
                        lhsT = kTp[g][32 * j : 32 * j + 32, m * C : (m + 1) * C]
                        for s in (0, 512):
                            mm(st_ps[:, s : s + 512], lhsT,
                               qTp[g][32 * j : 32 * j + 32, s : s + 512],
                               tile_position=(32 * j, 0))
                        pt = work.tile([C, L], BF16, tag="pt", bufs=3)
                        nc.scalar.activation(pt, st_ps, AF.Exp, bias=zero_t)
                        for s in (0, 512):
                            mm(o_ps[32 * j : 32 * j + 32, s : s + 512],
                               vaug[:, m, h, :], pt[:, s : s + 512],
                               start=(m == 0), stop=(m == 7),
                               tile_position=(0, 32 * j))
                nc.vector.tensor_copy(oTp[g], o_ps)
                # broadcast each head's sums row across its 32-row block,
                # then normalize in place: oTp = oTp / sums
                sb_ps = psum.tile([C, L], F32, tag="big", bufs=3, name="sb_ps")
                for j in range(4):
                    for s in (0, 512):
                        mm(sb_ps[32 * j : 32 * j + 32, s : s + 512],
                           E32[32 * j : 32 * j + 32, :],
                           oTp[g][32 * j : 32 * j + 32, s : s + 512],
                           tile_position=(32 * j, 32 * j))
                rT = work.tile([C, L], F32, tag="rT", bufs=1)
                nc.vector.reciprocal(rT, sb_ps)
                nc.vector.tensor_mul(oTp[g], oTp[g], rT)
            pr_ps = psum.tile([C, L], F32, tag="big", bufs=3)
            for s in (0, 512):
                for g in range(2):
                    mm(pr_ps[:, s : s + 512], proj_wT[stage][:, g, :],
                       oTp[g][:, s : s + 512],
                       start=(g == 0), stop=(g == 1))
            evac(dst, pr_ps, proj_b[stage])

        # ---------------- qv projection ----------------
        def qv_proj(stage, nx):
            qTp = [work.tile([C, L], BF16, tag="qT", bufs=2, name=f"qTp{g}")
                   for g in range(2)]
            vT = work.tile([C, L], BF16, tag="vT", bufs=1)
            for mc, dstt in ((0, qTp[0]), (1, qTp[1]), (2, vT)):
                qv_ps = psum.tile([C, L], F32, tag="big", bufs=3)
                for s in (0, 512):
                    mm(qv_ps[:, s : s + 512],
                       qv_wT[stage][:, mc * C : (mc + 1) * C], nx[:, s : s + 512])
                evac(dstt, qv_ps, qv_b[stage][:, mc : mc + 1])
            return qTp, vT

        # ---------------- mixffn ----------------
        def mixffn(m, nx, skip, add_src, dst):
            diag = diag_pool.tile([C, 36, C], BF16, tag="diag")
            nc.sync.dma_start(diag[:, 0:18, :],
                              D[f"diag_{m}"].rearrange("t p m -> p t m")[:, 0:18, :])
            nc.sync.dma_start(diag[:, 18:36, :],
                              D[f"diag_{m}"].rearrange("t p m -> p t m")[:, 18:36, :])
            ax_tiles = []
            for mc in range(4):
                h1_ps = psum.tile([C, L], F32, tag="big", bufs=3)
                for s in (0, 512):
                    mm(h1_ps[:, s : s + 512],
                       fc1_wT[m][:, mc * C : (mc + 1) * C], nx[:, s : s + 512])
                pad = work.tile([C, 34, 34], BF16, tag="h1pad", bufs=2)
                nc.gpsimd.memset(pad, 0.0)
                evac(pad[:, 1:33, 1:33],
                     h1_ps.rearrange("p (a b) -> p a b", a=HH),
                     fc1_b[m][:, mc : mc + 1])
                dw_ps = psum.tile([C, L], F32, tag="big", bufs=3)
                for t in range(9):
                    ky, kx = t // 3, t % 3
                    for half in (0, 1):
                        rhs = pad[:, ky + 16 * half : ky + 16 * half + 16,
                                  kx : kx + HH]
                        mm(dw_ps[:, half * 512 : half * 512 + 512],
                           diag[:, mc * 9 + t, :], rhs,
                           start=(t == 0), stop=(t == 8))
                if not skip:
                    ax = work.tile([C, L], BF16, tag="ax", bufs=4)
                    nc.scalar.activation(ax, dw_ps, AF.Gelu,
                                         bias=dw_b[m][:, mc : mc + 1])
                    ax_tiles.append(ax)
                else:
                    ssb = work.tile([C, L], F32, tag="ax", bufs=4, name="ssb")
                    nc.vector.tensor_scalar(ssb, dw_ps, dw_b[m][:, mc : mc + 1],
                                            None, op0=OP.add)
                    sv = ssb.rearrange("p (a b) -> p a b", a=HH)
                    nc.vector.tensor_add(sv, sv, pad[:, 1:33, 1:33])
                    ax_tiles.append(ssb)
            if skip:
                # LN over the 512 hidden channels, gelu folded in, cast to
                # bf16 chunks for fc2
                ax2 = [work.tile([C, L], BF16, tag="ax2", bufs=4,
                                 name=f"ax2_{mc}") for mc in range(4)]
                outs = [dict(dst_fn=sl2(ax2[mc]), act=AF.Gelu,
                             w=mln_w[:, mc : mc + 1], b=mln_b[:, mc : mc + 1])
                        for mc in range(4)]
                ln_cmajor([(C, sl2(t)) for t in ax_tiles], inv512, outs, L)
                ax_tiles = ax2
            mlp_ps = psum.tile([C, L], F32, tag="big", bufs=3)
            for s in (0, 512):
                for kc in range(4):
                    mm(mlp_ps[:, s : s + 512], fc2_wT[m][:, kc, :],
                       ax_tiles[kc][:, s : s + 512],
                       start=(kc == 0), stop=(kc == 3))
            t = work.tile([C, L], F32, tag="mlpout", bufs=1)
            evac(t, mlp_ps, fc2_b[m])
            nc.vector.tensor_add(dst, add_src, t)

        # ================== main flow ==================
        xcm = work.tile([C, L], F32, tag="resid", bufs=2, name="xcm")
        nc.sync.dma_start(xcm, D["x_in"])
        fore_sb = work.tile([FORE, 4 * L], F32, tag="fore_sb", bufs=1)
        nc.sync.dma_start(fore_sb, D["fore_in"])
        fore_pad = work.tile([FORE, 66, 66], BF16, tag="fore_pad", bufs=1)
        nc.gpsimd.memset(fore_pad, 0.0)
        post_sb = work.tile([C, 2, L // 4], F32, tag="post_sb", bufs=1)
        nc.sync.dma_start(post_sb, D["post_in"].rearrange("(k p) n -> p k n", p=C))

        # ---- stage 1 ----
        nx1 = work.tile([C, L], BF16, tag="nx", bufs=1)
        ln_cmajor([(C, sl2(xcm))], inv128,
                  [dict(dst_fn=sl2(nx1), w=ln_w[1], b=ln_b[1])], L)

        def fch_out(st, wdt):  # strip view of the padded fore interior
            r0 = st // 64
            return fore_pad[:, 1 + r0 : 1 + r0 + wdt // 64, 1:65]

        ln_cmajor([(FORE, sl2(fore_sb))], inv64,
                  [dict(dst_fn=fch_out, w=fore_ln_w, b=fore_ln_b)], 4 * L)

        # fore K conv (3x3, stride 2, pad 1) -> head-padded kfT [2][128, 1024]
        kfT = [work.tile([C, L], BF16, tag="kT", bufs=2, name=f"kfT{g}")
               for g in range(2)]
        for g in range(2):
            kf_ps = psum.tile([C, L], F32, tag="big", bufs=3, name="kf_ps")
            for t in range(9):
                ky, kx = t // 3, t % 3
                for half in (0, 1):
                    rhs = fore_pad[:, ky + 32 * half : ky + 32 * half + 32 : 2,
                                   kx : kx + 64 : 2]
                    mm(kf_ps[:, half * 512 : half * 512 + 512],
                       fore_taps[:, t, g * C : (g + 1) * C], rhs,
                       start=(t == 0), stop=(t == 8))
            evac(kfT[g], kf_ps, fore_k_b[:, g : g + 1])

        qT1, vT1 = qv_proj(1, nx1)
        attn1 = work.tile([C, L], F32, tag="attn", bufs=1)
        attention(1, qT1, vT1, kfT, attn1)
        add1 = work.tile([C, L], F32, tag="resid", bufs=2, name="add1")
        nc.vector.tensor_add(add1, xcm, attn1)

        nx2 = work.tile([C, L], BF16, tag="nx", bufs=1)
        ln_cmajor([(C, sl2(add1))], inv128,
                  [dict(dst_fn=sl2(nx2), w=ln_w[2], b=ln_b[2])], L)
        xt2 = work.tile([C, L], F32, tag="resid", bufs=2, name="xt2")
        mixffn(1, nx2, False, add1, xt2)

        # ---- stage 2 ----
        nx3 = work.tile([C, L], BF16, tag="nx", bufs=1)
        ln_cmajor([(C, sl2(xt2))], inv128,
                  [dict(dst_fn=sl2(nx3), w=ln_w[3], b=ln_b[3])], L)

        np0 = work.tile([C, L // 4], BF16, tag="npost", bufs=2)
        np1 = work.tile([C, L // 4], BF16, tag="npost", bufs=2, name="np1")
        ln_cmajor(
            [(C, lambda st, w: post_sb[:, 0, st : st + w]),
             (C, lambda st, w: post_sb[:, 1, st : st + w])], inv256,
            [dict(dst_fn=sl2(np0), w=post_ln_w[:, 0:1], b=post_ln_b[:, 0:1]),
             dict(dst_fn=sl2(np1), w=post_ln_w[:, 1:2], b=post_ln_b[:, 1:2])],
            L // 4)

        # post ConvT (2x2, stride 2) -> head-padded kpT [2][128, 1024]
        kpT = [work.tile([C, L], BF16, tag="kT", bufs=2, name=f"kpT{g}")
               for g in range(2)]
        for g in range(2):
            kpT_v = kpT[g].rearrange("p (y x) -> p y x", y=HH)
            for q in range(4):
                k_, l_ = q // 2, q % 2
                kp_ps = psum.tile([C, L // 4], F32, tag="big", bufs=3, name="kp_ps")
                for kc in range(2):
                    mm(kp_ps, post_taps[:, q * 2 + kc, g * C : (g + 1) * C],
                       np0 if kc == 0 else np1, start=(kc == 0), stop=(kc == 1))
                evac(kpT_v[:, k_ : HH : 2, l_ : HH : 2],
                     kp_ps.rearrange("p (i j) -> p i j", i=16),
                     post_k_b[:, g : g + 1])

        qT2, vT2 = qv_proj(2, nx3)
        attn2 = work.tile([C, L], F32, tag="attn", bufs=1, name="attn2")
        attention(2, qT2, vT2, kpT, attn2)
        add3 = work.tile([C, L], F32, tag="resid", bufs=2, name="add3")
        nc.vector.tensor_add(add3, xt2, attn2)

        nx4 = work.tile([C, L], BF16, tag="nx", bufs=1)
        ln_cmajor([(C, sl2(add3))], inv128,
                  [dict(dst_fn=sl2(nx4), w=ln_w[4], b=ln_b[4])], L)
        y_sb = work.tile([C, L], F32, tag="resid", bufs=2, name="y_sb")
        mixffn(2, nx4, True, add3, y_sb)

        nc.sync.dma_start(y_out, y_sb)


# --------------------------------------------------------------------------
# public entry point
# --------------------------------------------------------------------------
def _get_program():
    if "nc" not in _CACHE:
        _CACHE["nc"] = _build_program()
    return _CACHE["nc"]


def make_in_maps(x, fore_x, post_x, params):
    x = np.asarray(x, dtype=np.float32)
    fore_x = np.asarray(fore_x, dtype=np.float32)
    post_x = np.asarray(post_x, dtype=np.float32)
    w = _prep_weights(params)
    in_maps = []
    for b in range(B):
        m = {
            "x_in": np.ascontiguousarray(x[b].reshape(C, L)),
            "fore_in": np.ascontiguousarray(fore_x[b].reshape(FORE, 4 * L)),
            "post_in": np.ascontiguousarray(post_x[b].reshape(POST, L // 4)),
        }
        m.update(w)
        in_maps.append(m)
    return in_maps


def kernel(x, fore_x, post_x, params, trace=False):
    from concourse.bass_utils import run_bass_kernel_spmd

    nc = _get_program()
    in_maps = make_in_maps(x, fore_x, post_x, params)
    res = run_bass_kernel_spmd(nc, in_maps, core_ids=list(range(B)),
                               trace=trace)
    if trace:
        kernel.last_results = res
    out = np.stack([r["y_out"].reshape(C, HH, HH) for r in res.results])
    return out
